# revision 22
# baseline (speedup 1.0000x reference)
"""Trainium2 Bass kernel for nn_MemoryCore (retrieval KNN min-distance).

Problem: embedding [8192, 512], memory_bank [65536, 512] (fp32) ->
patch_scores [8192, 1] = min over the bank of euclidean distance.

Strategy (8 NeuronCores, SPMD):
  - Shard the memory bank (M axis) 8 ways; every core sees all queries.
  - fp8(e4m3) DoubleRow matmuls (contraction 256/instr, 2 fp8/cycle stream
    = the fp8 PE peak, ~215ns per K=256xN=512 matmul): psum[m, n] =
    (-2*bank_shard) @ emb.T. Bank tile stationary, reused across G=2 query
    blocks filling a 2-bank psum tile [128, 1024] (4 bufs, filled one unit
    at a time to keep 4-deep fill/evac pipelining).
  - Two query groups are interleaved unit-by-unit so the V-path bf16 min
    runs once per mt at FD=2048 across both groups' adjacent columns.
  - PSUM evacuation is split across both PSUM-capable engines so neither
    exceeds the PE's ~440us of matmul streaming (measured per-op costs:
    stt-from-PSUM 1283ns@1024, ACTIVATE 1111ns@1024, bf16 TT ~1223ns@2048):
      D: DVE rm = min(psum + m_sq[m], rm)      (fused stt, 1x from PSUM)
      V: ACT tmp = bf16(psum + m_sq[m]); DVE rm = min(tmp, rm) (2x bf16)
    bf16 mins are emitted LAG pairs late so a slow ACT never blocks
    PSUM-critical stt ops at the head of the DVE's strict FIFO.
  - No device epilogue: ship the bf16 running mins; host does the
    cross-partition + cross-core min, adds ||x||^2, sqrt.
"""
import numpy as np
import ml_dtypes
import concourse.bacc as bacc
import concourse.mybir as mybir
import concourse.tile as tile
from concourse.bass_utils import run_bass_kernel_spmd

N_CORES = 8
N, M, D = 8192, 65536, 512
MS = M // N_CORES       # 8192 bank rows per core
MT = MS // 128          # 64 bank tiles (psum partition dim)
G = 2                   # query blocks (512 each) sharing one weight load
GW = 512 * G            # 1024 queries per group
NPAIR = N // (2 * GW)   # 4 interleaved group pairs
BIG = 1e30
DT = mybir.dt.float8e4  # e4m3 (TRN variant, max +-240): 2x PE with DoubleRow
# Single-sided D-path: 1 of every 4 units takes the fused DVE stt (evenly
# spaced, never back-to-back); the rest go ACT -> lazy bf16 DVE min.
LAG = 3                 # pending bf16 mins held back this many entries

_CACHE = {}


def _build_kernel():
    nc = bacc.Bacc("TRN2", target_bir_lowering=False, debug=False,
                   num_devices=N_CORES)

    embT_d = nc.dram_tensor("embT", [D, N], DT, kind="ExternalInput")
    bankT_d = nc.dram_tensor("bankT", [D, MS], DT, kind="ExternalInput")
    msq_d = nc.dram_tensor("msq", [128, MT], mybir.dt.float32, kind="ExternalInput")
    outv_d = nc.dram_tensor("outv", [128, N], mybir.dt.bfloat16,
                            kind="ExternalOutput")

    PW = 2 * GW  # 2048 queries per group pair
    dma_engines = [nc.sync, nc.scalar, nc.gpsimd, nc.sync]

    with tile.TileContext(nc) as tc:
        with (
            tc.tile_pool(name="persist", bufs=1) as persist,
            tc.tile_pool(name="tmp", bufs=8) as tmpp,
            tc.tile_pool(name="psum", bufs=4, space="PSUM") as psum,
        ):
            msq = persist.tile([128, MT], mybir.dt.float32, tag="msq")
            nc.sync.dma_start(msq[:], msq_d[:])

            bank_t = persist.tile([128, 4, MS], DT, tag="bank")
            emb_t = persist.tile([128, 4, N], DT, tag="emb")
            # head-latency order: the first 8 mts' bank columns (0.5MB) and
            # pair 0's emb slice land first so matmuls start after ~1.5MB;
            # bank remainders ride the scalar/gpsimd rings in parallel.
            for k in range(4):
                nc.scalar.dma_start(bank_t[:, k, 0:1024],
                                    bankT_d[k * 128:(k + 1) * 128, 0:1024])
            nc.sync.dma_start(emb_t[:, 0, 0:PW], embT_d[0:128, 0:PW])
            nc.sync.dma_start(emb_t[:, 1, 0:PW], embT_d[128:256, 0:PW])
            nc.gpsimd.dma_start(emb_t[:, 2, 0:PW], embT_d[256:384, 0:PW])
            nc.gpsimd.dma_start(emb_t[:, 3, 0:PW], embT_d[384:512, 0:PW])
            for k in range(4):
                dma_engines[k].dma_start(bank_t[:, k, 1024:MS],
                                         bankT_d[k * 128:(k + 1) * 128,
                                                 1024:MS])
            for h in range(1, NPAIR):
                for k in range(4):
                    nc.sync.dma_start(
                        emb_t[:, k, h * PW:(h + 1) * PW],
                        embT_d[k * 128:(k + 1) * 128, h * PW:(h + 1) * PW])

            rm_t = [persist.tile([128, PW], mybir.dt.bfloat16,
                                 name=f"rm{h}", tag=f"rm{h}")
                    for h in range(NPAIR)]

            for h in range(NPAIR):
                rm = rm_t[h]
                nc.gpsimd.memset(rm[:], BIG)
                # pending bf16 mins: (tmp tile AP, rm slice AP), emitted late
                pending = []
                t = None
                for mt in range(MT):
                    for u in range(2):
                        g = 2 * h + u
                        ps = psum.tile([128, GW], mybir.dt.float32, tag="ps")
                        for kp in range(2):
                            w = bank_t[:, kp * 2:(kp + 1) * 2,
                                       mt * 128:(mt + 1) * 128]
                            for j in range(G):
                                nb = g * G + j
                                nc.tensor.matmul(
                                    ps[:, j * 512:(j + 1) * 512],
                                    w,
                                    emb_t[:, kp * 2:(kp + 1) * 2,
                                          nb * 512:(nb + 1) * 512],
                                    start=(kp == 0),
                                    stop=(kp == 1),
                                    perf_mode=mybir.MatmulPerfMode.DoubleRow,
                                )
                        # single-sided D: one stt per D-mt (u0 on mt%4==0,
                        # u1 on mt%4==2) so stts never burst back-to-back
                        is_d = (u == 0 and mt % 4 == 0) or \
                               (u == 1 and mt % 4 == 2)
                        rm_half = rm[:, u * GW:(u + 1) * GW]
                        if is_d:
                            # rm = min(psum + m_sq[m], rm)  (DVE, 1x from PSUM)
                            nc.vector.scalar_tensor_tensor(
                                out=rm_half,
                                in0=ps[:],
                                scalar=msq[:, mt:mt + 1],
                                in1=rm_half,
                                op0=mybir.AluOpType.add,
                                op1=mybir.AluOpType.min,
                            )
                        elif mt % 4 in (1, 3):
                            # both halves V: share one tmp pair + one TT@2048
                            if u == 0:
                                t = tmpp.tile([128, PW], mybir.dt.bfloat16,
                                              tag="t2")
                            nc.scalar.activation(
                                out=t[:, u * GW:(u + 1) * GW], in_=ps[:],
                                func=mybir.ActivationFunctionType.Identity,
                                bias=msq[:, mt:mt + 1],
                            )
                            if u == 1:
                                pending.append((t, rm))
                        else:
                            # partner half of a D-mt: own tmp + TT@1024
                            t1 = tmpp.tile([128, GW], mybir.dt.bfloat16,
                                           tag="t1")
                            nc.scalar.activation(
                                out=t1[:], in_=ps[:],
                                func=mybir.ActivationFunctionType.Identity,
                                bias=msq[:, mt:mt + 1],
                            )
                            pending.append((t1, rm_half))
                        lag = 0 if (h == NPAIR - 1 and mt >= MT - 4) else LAG
                        while len(pending) > lag:
                            tp, dst = pending.pop(0)
                            nc.vector.tensor_tensor(
                                out=dst, in0=tp[:], in1=dst,
                                op=mybir.AluOpType.min)
                for tp, dst in pending:
                    nc.vector.tensor_tensor(
                        out=dst, in0=tp[:], in1=dst,
                        op=mybir.AluOpType.min)
                nc.sync.dma_start(outv_d[:, h * PW:(h + 1) * PW], rm[:])

    nc.compile()
    return nc


def kernel(embedding: np.ndarray, memory_bank: np.ndarray) -> np.ndarray:
    emb = np.asarray(embedding, dtype=np.float32)
    bank = np.asarray(memory_bank, dtype=np.float32)
    assert emb.shape == (N, D) and bank.shape == (M, D)

    if "nc" not in _CACHE:
        _CACHE["nc"] = _build_kernel()
    nc = _CACHE["nc"]

    embT8 = np.ascontiguousarray(emb.T).astype(ml_dtypes.float8_e4m3)
    x_sq = np.einsum("nd,nd->n", emb, emb, dtype=np.float64)  # [N]

    in_maps = []
    for c in range(N_CORES):
        shard = bank[c * MS:(c + 1) * MS]
        bankT8 = np.ascontiguousarray((-2.0 * shard).T).astype(
            ml_dtypes.float8_e4m3)
        m_sq = np.einsum("md,md->m", shard, shard,
                         dtype=np.float64).astype(np.float32)
        msq = np.ascontiguousarray(m_sq.reshape(MT, 128).T)
        in_maps.append({"embT": embT8, "bankT": bankT8, "msq": msq})

    _CACHE["last_in_maps"] = in_maps
    try:
        res = run_bass_kernel_spmd(nc, in_maps, core_ids=list(range(N_CORES)))
    except Exception:
        # a previously-wedged NeuronCore reports unrecoverable once and then
        # recovers; one retry clears it
        import time
        time.sleep(2.0)
        res = run_bass_kernel_spmd(nc, in_maps, core_ids=list(range(N_CORES)))

    # gather: each core returns [128, N] bf16 partial mins of (m_sq - 2 x.m);
    # min over partitions and cores, then + ||x||^2 and sqrt on host.
    per_core = np.stack([
        res.results[c]["outv"].astype(np.float64).min(axis=0)
        for c in range(N_CORES)
    ])  # [8, N]
    tot = per_core.min(axis=0) + x_sq
    return np.sqrt(np.maximum(tot, 0.0)).astype(np.float32).reshape(N, 1)
